# revision 1
# baseline (speedup 1.0000x reference)
"""DisenGCN (Zinc) forward pass on 8 Trainium2 NeuronCores.

Strategy (node-partitioned, edge-local):
  - Sort edges by trg; split nodes into 8 contiguous, graph-aligned ranges with
    balanced edge counts. Each core owns its node range and every edge whose trg
    falls in it, so all 3 routing iterations per layer are communication-free.
  - Per layer: l2-normalize the local shard, AllGather the normalized features
    (bf16) so every core can gather z = x[src] for its edges (one dma_gather per
    layer), then run 3 local routing iterations.
  - Scatter (segment_sum by trg) and gather (u[trg]) are both expressed as
    128x128 one-hot matmuls on the tensor engine; the one-hots are host-built
    (trg is static) and streamed from HBM in bf16.
  - Per-edge softmax work runs on DVE/ACT with edges on partitions (GPSIMD
    measured ~4.3us/op for the s=z*p mul and bottlenecked the kernel, so all
    elementwise stays on DVE; the capsule-dot reduce is two-stage: a bf16 2x
    halving add, then a 1x reduce over the half).
  - Readout (per-graph mean + 2-layer MLP) is computed on-device per core for
    its own (whole) graphs; host concatenates the [G,1] output.
"""

import os
import sys
import time

sys.path.insert(0, "/opt/trn_rl_repo")

import numpy as np
import ml_dtypes

import concourse.bass as bass
import concourse.bacc as bacc
import concourse.tile as tile
import concourse.mybir as mybir

NCORES = 8
D = 128
NLAYER = 4
ROUTIT = 3
TAU = 1.0
BN_EPS = 1e-5
KS = (8, 8, 4, 4)
GSZ = 8           # chunks (of 128 edges) per processing group
NPIECE = 8        # dma_gather pieces per layer (overlap with compute)
GPSIMD_S = os.environ.get("DGZ_GPSIMD_S", "0") == "1"

F32 = mybir.dt.float32
BF16 = mybir.dt.bfloat16
I16 = mybir.dt.int16
AX = mybir.AxisListType
ALU = mybir.AluOpType
ACTF = mybir.ActivationFunctionType


# --------------------------------------------------------------------------
# host preprocessing
# --------------------------------------------------------------------------

def _wrap16(idx):
    """[n] int -> [128, n/16] int16 in the dma_gather wrapped layout."""
    n = len(idx)
    assert n % 16 == 0
    a = np.asarray(idx).reshape(n // 16, 16).T.astype(np.int16)  # [16, cols]
    return np.tile(a, (8, 1))  # [128, cols]


def preprocess(inputs, ncores=NCORES, g_out=None):
    x = np.asarray(inputs["x"]).astype(np.int64)
    src = np.asarray(inputs["src"]).astype(np.int64)
    trg = np.asarray(inputs["trg"]).astype(np.int64)
    snorm_n = np.asarray(inputs["snorm_n"]).astype(np.float32)
    gid = np.asarray(inputs["gid"]).astype(np.int64)
    N = x.shape[0]
    M = src.shape[0]
    if g_out is None:
        g_out = 512
    G = g_out

    # graph -> node-range starts (gid is sorted)
    gstarts = np.searchsorted(gid, np.arange(G + 1))  # [G+1]
    deg = np.bincount(trg, minlength=N)
    cume = np.concatenate([[0], np.cumsum(deg)])      # cume[n] = #edges with trg < n
    edges_at_gstart = cume[gstarts]                   # [G+1]

    # split graphs across cores balancing edge counts
    gsplit = np.zeros(ncores + 1, np.int64)
    gsplit[ncores] = G
    for c in range(1, ncores):
        t = M * c / ncores
        g = int(np.searchsorted(edges_at_gstart, t))
        if g > 0 and abs(edges_at_gstart[g - 1] - t) <= abs(edges_at_gstart[min(g, G)] - t):
            g = g - 1
        gsplit[c] = min(max(g, gsplit[c - 1] + 1), G - (ncores - c))
    node_split = gstarts[gsplit]

    max_nodes = int(np.max(np.diff(node_split)))
    NBLK = (max_nodes + 127) // 128
    NODE_PAD = NBLK * 128

    edge_order = np.argsort(trg, kind="stable")
    strg = trg[edge_order]
    ssrc = src[edge_order]

    # per-core bin packing of nodes into NBLK bins of <=128 nodes, balancing edges
    owner = np.zeros(N, np.int32)
    slot_of = np.zeros(N, np.int64)  # local slot within the core
    cores = []
    NCB = 1
    for c in range(ncores):
        nlo, nhi = int(node_split[c]), int(node_split[c + 1])
        nodes = np.arange(nlo, nhi)
        nd = deg[nlo:nhi]
        order = np.argsort(-nd, kind="stable")
        bin_load = np.zeros(NBLK, np.int64)
        bin_cnt = np.zeros(NBLK, np.int64)
        bin_of = np.zeros(nhi - nlo, np.int32)
        pos_in = np.zeros(nhi - nlo, np.int32)
        for i in order:
            cand = np.where(bin_cnt < 128)[0]
            b = cand[np.argmin(bin_load[cand])]
            bin_of[i] = b
            pos_in[i] = bin_cnt[b]
            bin_cnt[b] += 1
            bin_load[b] += nd[i]
        slots = bin_of.astype(np.int64) * 128 + pos_in
        owner[nodes] = c
        slot_of[nodes] = slots
        NCB = max(NCB, int(np.max((bin_load + 127) // 128)))
        cores.append(dict(nlo=nlo, nhi=nhi, glo=int(gsplit[c]), ghi=int(gsplit[c + 1]),
                          bin_load=bin_load))
    padded_id = owner.astype(np.int64) * NODE_PAD + slot_of  # global padded id

    TOTCH = NBLK * NCB
    NGR = (TOTCH + GSZ - 1) // GSZ

    # shared parameter folds
    emb = np.asarray(inputs["embed_table"]).astype(np.float32)
    t_tab = emb @ np.asarray(inputs["pca_w"]).astype(np.float32) \
        + np.asarray(inputs["pca_b"]).astype(np.float32)          # [28, 128]
    bn_g = np.asarray(inputs["bn_gamma"]).astype(np.float32)
    bn_b = np.asarray(inputs["bn_beta"]).astype(np.float32)
    bn_m = np.asarray(inputs["bn_mean"]).astype(np.float32)
    bn_v = np.asarray(inputs["bn_var"]).astype(np.float32)
    A = bn_g / np.sqrt(bn_v + BN_EPS)                              # [4, 128]
    B = bn_b - bn_m * A
    bn_a_rep = np.repeat(A[:, None, :], 128, axis=1).astype(np.float32)  # [4,128,128]
    bn_b_rep = np.repeat(B[:, None, :], 128, axis=1).astype(np.float32)

    gcnt_all = np.bincount(gid, minlength=G).astype(np.float32)

    shared = {
        "t_tab": np.ascontiguousarray(t_tab),
        "bn_a": bn_a_rep, "bn_b": bn_b_rep,
        "w1": np.asarray(inputs["reg1_w"]).astype(np.float32),
        "b1": np.asarray(inputs["reg1_b"]).astype(np.float32).reshape(1, -1),
        "w2": np.asarray(inputs["reg2_w"]).astype(np.float32),
        "b2": np.asarray(inputs["reg2_b"]).astype(np.float32).reshape(1, 1),
        "ones1": np.ones((1, 128), np.float32),
        "ident": np.eye(128, dtype=np.float32),
    }

    per_core = []
    for c in range(ncores):
        cc = cores[c]
        nlo, nhi, glo, ghi = cc["nlo"], cc["nhi"], cc["glo"], cc["ghi"]
        gcnt = ghi - glo
        assert gcnt <= 128, f"core {c} owns {gcnt} graphs > 128"
        n_c = nhi - nlo

        # local edges (sorted-by-trg slice)
        elo, ehi = int(cume[nlo]), int(cume[nhi])
        etrg = strg[elo:ehi]
        esrc = ssrc[elo:ehi]
        eslot = slot_of[etrg]                   # local slot of target
        ebin = (eslot // 128).astype(np.int64)
        ecol = (eslot % 128).astype(np.int64)
        # order edges by bin (stable)
        eord = np.argsort(ebin, kind="stable")
        ebin = ebin[eord]; ecol = ecol[eord]; esrc2 = esrc[eord]
        # position within bin
        bin_edge_cnt = np.bincount(ebin, minlength=NBLK)
        assert int(np.max(bin_edge_cnt)) <= NCB * 128
        bin_first = np.concatenate([[0], np.cumsum(bin_edge_cnt)])[:-1]
        within = np.arange(len(ebin)) - bin_first[ebin]
        chunk = ebin * NCB + within // 128
        row = within % 128

        O = np.zeros((TOTCH, 128, 128), np.float32)
        O[chunk, row, ecol] = 1.0
        O = O.astype(ml_dtypes.bfloat16)
        OT = np.ascontiguousarray(O.transpose(0, 2, 1))

        def group(o):
            pad = NGR * GSZ - TOTCH
            if pad:
                o = np.concatenate([o, np.zeros((pad, 128, 128), o.dtype)], 0)
            # [NGR, GSZ, 128, 128] -> [NGR, 128, GSZ*128]
            return np.ascontiguousarray(
                o.reshape(NGR, GSZ, 128, 128).transpose(0, 2, 1, 3).reshape(NGR, 128, GSZ * 128))

        o_grp = group(O)
        ot_grp = group(OT)

        src_pad = np.zeros(TOTCH * 128, np.int64)
        src_pad[chunk * 128 + row] = padded_id[esrc2]
        src_idx = _wrap16(src_pad)

        l2g = np.zeros(NODE_PAD, np.int64)
        real = np.zeros(NODE_PAD, bool)
        nodes = np.arange(nlo, nhi)
        l2g[slot_of[nodes]] = nodes
        real[slot_of[nodes]] = True

        x_idx = _wrap16(np.where(real, x[l2g], 0))

        snorm = np.zeros((128, NBLK), np.float32)
        sl = slot_of[nodes]
        snorm[sl % 128, sl // 128] = snorm_n[nodes, 0]

        go = np.zeros((NBLK, 128, 128), np.float32)
        gcol = gid[nodes] - glo
        go[sl // 128, sl % 128, gcol] = 1.0
        go = go.astype(ml_dtypes.bfloat16)

        rc = np.zeros((128, 1), np.float32)
        rc[:gcnt, 0] = 1.0 / np.maximum(gcnt_all[glo:ghi], 1.0)

        per_core.append({
            "x_idx": x_idx, "src_idx": src_idx,
            "o_grp": o_grp, "ot_grp": ot_grp, "go_blk": go,
            "snorm": snorm, "rcnt": rc,
            "_glo": glo, "_ghi": ghi,
        })

    meta = dict(NBLK=NBLK, NCB=NCB, NODE_PAD=NODE_PAD, TOTCH=TOTCH, NGR=NGR,
                ncores=ncores, G=G)
    return meta, shared, per_core


# --------------------------------------------------------------------------
# bass program
# --------------------------------------------------------------------------

def build_program(meta, verbose=False):
    NBLK = meta["NBLK"]; NCB = meta["NCB"]; NODE_PAD = meta["NODE_PAD"]
    TOTCH = meta["TOTCH"]; NGR = meta["NGR"]; ncores = meta["ncores"]
    NALL = NODE_PAD * ncores

    nc = bacc.Bacc("TRN2", target_bir_lowering=False, debug=False,
                   num_devices=ncores)

    t_tab = nc.dram_tensor("t_tab", [28, D], F32, kind="ExternalInput")
    x_idx = nc.dram_tensor("x_idx", [128, NODE_PAD // 16], I16, kind="ExternalInput")
    src_idx = nc.dram_tensor("src_idx", [128, TOTCH * 8], I16, kind="ExternalInput")
    o_grp = nc.dram_tensor("o_grp", [NGR, 128, GSZ * 128], BF16, kind="ExternalInput")
    ot_grp = nc.dram_tensor("ot_grp", [NGR, 128, GSZ * 128], BF16, kind="ExternalInput")
    go_blk = nc.dram_tensor("go_blk", [NBLK, 128, 128], BF16, kind="ExternalInput")
    snorm_d = nc.dram_tensor("snorm", [128, NBLK], F32, kind="ExternalInput")
    bn_a_d = nc.dram_tensor("bn_a", [NLAYER, 128, 128], F32, kind="ExternalInput")
    bn_b_d = nc.dram_tensor("bn_b", [NLAYER, 128, 128], F32, kind="ExternalInput")
    rcnt_d = nc.dram_tensor("rcnt", [128, 1], F32, kind="ExternalInput")
    w1_d = nc.dram_tensor("w1", [D, 64], F32, kind="ExternalInput")
    b1_d = nc.dram_tensor("b1", [1, 64], F32, kind="ExternalInput")
    w2_d = nc.dram_tensor("w2", [64, 1], F32, kind="ExternalInput")
    b2_d = nc.dram_tensor("b2", [1, 1], F32, kind="ExternalInput")
    ones1_d = nc.dram_tensor("ones1", [1, 128], F32, kind="ExternalInput")
    ident_d = nc.dram_tensor("ident", [128, 128], F32, kind="ExternalInput")
    gout_d = nc.dram_tensor("gout", [128, 1], F32, kind="ExternalOutput")

    with tile.TileContext(nc) as tc:
        with (
            tc.tile_pool(name="dram", bufs=1, space="DRAM") as dram,
            tc.tile_pool(name="big", bufs=1) as big,
            tc.tile_pool(name="par", bufs=1) as par,
            tc.tile_pool(name="bnd", bufs=1) as bnd,
            tc.tile_pool(name="ogrp", bufs=4) as ogrp_pool,
            tc.tile_pool(name="grpw", bufs=4) as grpw,
            tc.tile_pool(name="sm", bufs=4) as smp,
            tc.tile_pool(name="node", bufs=3) as nodep,
            tc.tile_pool(name="ps_utrg", bufs=4, space="PSUM") as ps_utrg,
            tc.tile_pool(name="ps_ublk", bufs=3, space="PSUM") as ps_ublk,
            tc.tile_pool(name="ps_ro", bufs=1, space="PSUM") as ps_ro,
        ):
            v = nc.vector
            sc = nc.scalar
            gp = nc.gpsimd
            pe = nc.tensor

            # --- persistent SBUF ---
            z_sb = big.tile([128, TOTCH, D], BF16, tag="z")
            u_bf = big.tile([128, NBLK, D], BF16, tag="ubf")
            xbuf = big.tile([128, NBLK, D], F32, tag="xbuf")
            hA = big.tile([128, NBLK, D], F32, tag="hA")
            hB = big.tile([128, NBLK, D], F32, tag="hB")
            xidx_sb = par.tile([128, NODE_PAD // 16], I16, tag="xidx")
            sidx_sb = par.tile([128, TOTCH * 8], I16, tag="sidx")
            snorm_sb = par.tile([128, NBLK], F32, tag="snorm")
            rcnt_sb = par.tile([128, 1], F32, tag="rcnt")
            w1_sb = par.tile([D, 64], F32, tag="w1")
            b1_sb = par.tile([1, 64], F32, tag="b1")
            w2_sb = par.tile([64, 1], F32, tag="w2")
            b2_sb = par.tile([1, 1], F32, tag="b2")
            ones_sb = par.tile([1, 128], F32, tag="ones")
            ident_sb = par.tile([128, 128], F32, tag="ident")

            nc.sync.dma_start(xidx_sb[:], x_idx[:, :])
            nc.sync.dma_start(sidx_sb[:], src_idx[:, :])
            nc.sync.dma_start(snorm_sb[:], snorm_d[:, :])
            nc.sync.dma_start(rcnt_sb[:], rcnt_d[:, :])
            nc.sync.dma_start(w1_sb[:], w1_d[:, :])
            nc.sync.dma_start(b1_sb[:], b1_d[:, :])
            nc.sync.dma_start(w2_sb[:], w2_d[:, :])
            nc.sync.dma_start(b2_sb[:], b2_d[:, :])
            nc.sync.dma_start(ones_sb[:], ones1_d[:, :])
            nc.sync.dma_start(ident_sb[:], ident_d[:, :])

            # --- collective bounce buffers (ping-pong across layers) ---
            cc_in = [dram.tile([NODE_PAD, D], BF16, tag=f"cci{i}", name=f"cc_in{i}")
                     for i in range(NLAYER)]
            cc_space = "Shared" if ncores > 4 else "Local"
            cc_out = [dram.tile([NALL, D], BF16, tag=f"cco{i}", name=f"cc_out{i}",
                                addr_space=cc_space)
                      for i in range(NLAYER)]

            def fl(ap):
                return ap.rearrange("p a b -> p (a b)")

            def lrelu(out_ap, in_ap, eng=None):
                (eng or v).scalar_tensor_tensor(out_ap, in_ap, 0.01, in_ap,
                                                ALU.mult, ALU.max)

            # --- phase 0: embedding ---
            h0 = bnd.tile([128, NBLK, D], F32, tag="bndA")
            gp.dma_gather(out_ap=h0[:], in_ap=t_tab.ap(), idxs_ap=xidx_sb[:],
                          num_idxs=NODE_PAD, num_idxs_reg=NODE_PAD, elem_size=D,
                          single_packet=False)
            lrelu(fl(hA[:]), fl(h0[:]))

            h_cur = hA
            for layer in range(NLAYER):
                K = KS[layer]
                DD = D // K
                pin = layer
                # ---- l2norm of h -> xbuf (f32) + u_bf (bf16) ----
                sq = bnd.tile([128, NBLK, D], F32, tag="bndA")
                sc.square(fl(sq[:]), fl(h_cur[:]))
                ss = smp.tile([128, NBLK * K], F32, tag="ss_l")
                v.reduce_sum(ss[:], sq[:].rearrange("p b (k e) -> p (b k) e", k=K), axis=AX.X)
                nrm = smp.tile([128, NBLK * K], F32, tag="nrm_l")
                sc.sqrt(nrm[:], ss[:])
                nrm2 = smp.tile([128, NBLK * K], F32, tag="nrm2_l")
                v.tensor_scalar_max(nrm2[:], nrm[:], 1e-12)
                rno = smp.tile([128, NBLK * K], F32, tag="rno_l")
                v.reciprocal(rno[:], nrm2[:])
                v.tensor_mul(xbuf[:].rearrange("p b (k e) -> p (b k) e", k=K),
                             h_cur[:].rearrange("p b (k e) -> p (b k) e", k=K),
                             rno[:].unsqueeze(2).broadcast_to([128, NBLK * K, DD]))
                sc.copy(fl(u_bf[:]), fl(xbuf[:]))

                # ---- allgather x (bf16, row layout) ----
                nc.sync.dma_start(
                    cc_in[pin][:].rearrange("(b p) d -> p b d", p=128), u_bf[:])
                gp.collective_compute(
                    "AllGather", ALU.bypass,
                    replica_groups=[list(range(ncores))],
                    ins=[cc_in[pin][:].opt()], outs=[cc_out[pin][:].opt()],
                )

                # ---- z gather (pieces) ----
                piece = ((TOTCH + NPIECE - 1) // NPIECE + GSZ - 1) // GSZ * GSZ
                for a in range(0, TOTCH, piece):
                    b = min(a + piece, TOTCH)
                    gp.dma_gather(
                        out_ap=z_sb[:, a:b, :], in_ap=cc_out[pin][:],
                        idxs_ap=sidx_sb[:, a * 8:b * 8],
                        num_idxs=(b - a) * 128, num_idxs_reg=(b - a) * 128,
                        elem_size=D, single_packet=False)

                # ---- routing iterations ----
                for it in range(ROUTIT):
                    last = it == ROUTIT - 1
                    ub_ps = {}
                    for g0 in range(0, TOTCH, GSZ):
                        gs = min(GSZ, TOTCH - g0)
                        gi = g0 // GSZ
                        # gather u[trg] on PE
                        ot_sb = ogrp_pool.tile([128, GSZ * 128], BF16, tag="ot")
                        nc.sync.dma_start(ot_sb[:], ot_grp[gi])
                        H = GSZ // 2
                        utps0 = ps_utrg.tile([128, H * 128], F32, tag="utrg", name=f"utps0_{it}_{g0}")
                        utps1 = ps_utrg.tile([128, H * 128], F32, tag="utrg", name=f"utps1_{it}_{g0}")
                        utsb = grpw.tile([128, GSZ * 128], BF16, tag="utsb")
                        for j in range(gs):
                            ch = g0 + j
                            utps = utps0 if j < H else utps1
                            jo = j if j < H else j - H
                            pe.matmul(utps[:, jo * 128:(jo + 1) * 128],
                                      ot_sb[:, j * 128:(j + 1) * 128],
                                      u_bf[:, ch // NCB, :],
                                      start=True, stop=True)
                        h0n = min(gs, H) * 128
                        sc.copy(utsb[:, :h0n], utps0[:, :h0n])
                        if gs > H:
                            sc.copy(utsb[:, H * 128:gs * 128], utps1[:, :(gs - H) * 128])
                        prod = grpw.tile([128, GSZ * 128], BF16, tag="prod")
                        zsl = z_sb[:, g0:g0 + gs, :]
                        v.tensor_mul(prod[:, :gs * 128].rearrange("p (g e) -> p g e", g=gs),
                                     zsl, utsb[:, :gs * 128].rearrange("p (g e) -> p g e", g=gs))
                        ph = grpw.tile([128, GSZ * 64], BF16, tag="ph")
                        pr3 = prod[:, :gs * 128].rearrange(
                            "p (g k e) -> p (g k) e", g=gs, k=K)
                        v.tensor_add(ph[:, :gs * 64].rearrange(
                                         "p (g k e) -> p (g k) e", g=gs, k=K),
                                     pr3[:, :, :DD // 2], pr3[:, :, DD // 2:])
                        logits = smp.tile([128, GSZ * KS[layer]], F32, tag="log")
                        v.reduce_sum(logits[:, :gs * K],
                                     ph[:, :gs * 64].rearrange(
                                         "p (g k e) -> p (g k) e", g=gs, k=K),
                                     axis=AX.X)
                        eraw = smp.tile([128, GSZ * KS[layer]], F32, tag="eraw")
                        sc.activation(eraw[:, :gs * K], logits[:, :gs * K],
                                      ACTF.Exp, scale=1.0 / TAU)
                        sume = smp.tile([128, GSZ], F32, tag="sume")
                        v.reduce_sum(sume[:, :gs],
                                     eraw[:, :gs * K].rearrange("p (g k) -> p g k", g=gs),
                                     axis=AX.X)
                        rsum = smp.tile([128, GSZ], F32, tag="rsum")
                        v.reciprocal(rsum[:, :gs], sume[:, :gs])
                        pz = smp.tile([128, GSZ * KS[layer]], F32, tag="pz")
                        v.tensor_mul(pz[:, :gs * K].rearrange("p (g k) -> p g k", g=gs),
                                     eraw[:, :gs * K].rearrange("p (g k) -> p g k", g=gs),
                                     rsum[:, :gs].unsqueeze(2).broadcast_to([128, gs, K]))
                        s_sb = grpw.tile([128, GSZ * 128], BF16, tag="s")
                        s_eng = gp if (GPSIMD_S and hasattr(gp, "tensor_mul")) else v
                        s_eng.tensor_mul(
                            s_sb[:, :gs * 128].rearrange("p (g k e) -> p g k e", g=gs, k=K),
                            zsl.rearrange("p g (k e) -> p g k e", k=K),
                            pz[:, :gs * K].rearrange("p (g k) -> p g k", g=gs)
                                .unsqueeze(3).broadcast_to([128, gs, K, DD]))
                        # scatter on PE
                        o_sb = ogrp_pool.tile([128, GSZ * 128], BF16, tag="o")
                        nc.sync.dma_start(o_sb[:], o_grp[gi])
                        for j in range(gs):
                            ch = g0 + j
                            bin_ = ch // NCB
                            jj = ch % NCB
                            if jj == 0:
                                ub_ps[bin_] = ps_ublk.tile(
                                    [128, 128], F32, tag="ublk", name=f"ublk_b{bin_}_i{it}")
                            pe.matmul(ub_ps[bin_][:], o_sb[:, j * 128:(j + 1) * 128],
                                      s_sb[:, j * 128:(j + 1) * 128],
                                      start=(jj == 0), stop=(jj == NCB - 1))
                            if jj == NCB - 1:
                                # node phase for bin_
                                t0 = nodep.tile([128, 128], F32, tag="t0")
                                v.tensor_add(t0[:], ub_ps[bin_][:], xbuf[:, bin_, :])
                                sq2 = nodep.tile([128, 128], F32, tag="sq2")
                                sc.square(sq2[:], t0[:])
                                ss2 = nodep.tile([128, KS[layer]], F32, tag="ss2")
                                v.reduce_sum(ss2[:], sq2[:].rearrange("p (k e) -> p k e", k=K),
                                             axis=AX.X)
                                nr2 = nodep.tile([128, KS[layer]], F32, tag="nr2")
                                sc.sqrt(nr2[:], ss2[:])
                                nr2m = nodep.tile([128, KS[layer]], F32, tag="nr2m")
                                v.tensor_scalar_max(nr2m[:], nr2[:], 1e-12)
                                rn2 = nodep.tile([128, KS[layer]], F32, tag="rn2")
                                v.reciprocal(rn2[:], nr2m[:])
                                dst = (hB if layer % 2 == 0 else hA) if last else u_bf
                                v.tensor_mul(
                                    dst[:, bin_, :].rearrange("p (k e) -> p k e", k=K),
                                    t0[:].rearrange("p (k e) -> p k e", k=K),
                                    rn2[:].unsqueeze(2).broadcast_to([128, K, DD]))

                # ---- layer tail: h = lrelu((u*snorm)*A + B) ----
                # ping-pong through the single bnd temp to stay in budget:
                # m1(bnd) = u*snorm; h_new = m1*A; m3(bnd) = h_new+B; h_new = lrelu(m3)
                h_new = hB if layer % 2 == 0 else hA
                a_sb = bnd.tile([128, 128], F32, tag="a_sb")
                b_sb = bnd.tile([128, 128], F32, tag="b_sb")
                nc.sync.dma_start(a_sb[:], bn_a_d[layer])
                nc.sync.dma_start(b_sb[:], bn_b_d[layer])
                m1 = bnd.tile([128, NBLK, D], F32, tag="bndA")
                v.tensor_mul(m1[:], h_new[:],
                             snorm_sb[:].unsqueeze(2).broadcast_to([128, NBLK, D]))
                v.tensor_mul(h_new[:], m1[:],
                             a_sb[:].unsqueeze(1).broadcast_to([128, NBLK, D]))
                m3 = bnd.tile([128, NBLK, D], F32, tag="bndA")
                v.tensor_add(m3[:], h_new[:],
                             b_sb[:].unsqueeze(1).broadcast_to([128, NBLK, D]))
                lrelu(fl(h_new[:]), fl(m3[:]))
                h_cur = h_new

            # ---- readout ----
            h_bf = u_bf  # u_bf is dead after the last layer; reuse its slot
            sc.copy(fl(h_bf[:]), fl(h_cur[:]))
            gsum = ps_ro.tile([128, 128], F32, tag="ro")
            for b in range(NBLK):
                go_sb = ogrp_pool.tile([128, 128], BF16, tag="go")
                nc.sync.dma_start(go_sb[:], go_blk[b])
                pe.matmul(gsum[:], go_sb[:], h_bf[:, b, :],
                          start=(b == 0), stop=(b == NBLK - 1))
            g0t = nodep.tile([128, 128], F32, tag="g0")
            sc.activation(g0t[:], gsum[:], ACTF.Copy, scale=rcnt_sb[:, :])
            g0l = nodep.tile([128, 128], F32, tag="g0l")
            lrelu(g0l[:], g0t[:])
            tps = ps_ro.tile([128, 128], F32, tag="ro")
            pe.transpose(tps[:], g0l[:], ident_sb[:])
            g0T = nodep.tile([128, 128], F32, tag="g0T")
            sc.copy(g0T[:], tps[:])
            mm1 = ps_ro.tile([128, 64], F32, tag="ro")
            pe.matmul(mm1[:], g0T[:], w1_sb[:], start=True, stop=False)
            pe.matmul(mm1[:], ones_sb[:], b1_sb[:], start=False, stop=True)
            g1c = nodep.tile([128, 64], F32, tag="g1c")
            sc.copy(g1c[:], mm1[:])
            g1 = nodep.tile([128, 64], F32, tag="g1")
            lrelu(g1[:], g1c[:])
            tps2 = ps_ro.tile([64, 128], F32, tag="ro")
            pe.transpose(tps2[:], g1[:], ident_sb[:])
            g1T = nodep.tile([64, 128], F32, tag="g1T")
            sc.copy(g1T[:], tps2[:])
            mm2 = ps_ro.tile([128, 1], F32, tag="ro")
            pe.matmul(mm2[:], g1T[:], w2_sb[:], start=True, stop=False)
            pe.matmul(mm2[:], ones_sb[:], b2_sb[:], start=False, stop=True)
            gfin = nodep.tile([128, 1], F32, tag="gfin")
            sc.copy(gfin[:], mm2[:])
            nc.sync.dma_start(gout_d[:, :], gfin[:])

    t0 = time.time()
    nc.compile()
    if verbose:
        print(f"bacc compile: {time.time() - t0:.1f}s", flush=True)
    return nc


def make_in_maps(meta, shared, per_core):
    in_maps = []
    for c in range(meta["ncores"]):
        m = dict(shared)
        pc = per_core[c]
        m.update({k: v for k, v in pc.items() if not k.startswith("_")})
        in_maps.append(m)
    return in_maps


def assemble_output(meta, per_core, results):
    G = meta["G"]
    out = np.zeros((G, 1), np.float32)
    for c in range(meta["ncores"]):
        glo, ghi = per_core[c]["_glo"], per_core[c]["_ghi"]
        out[glo:ghi] = results[c]["gout"][:ghi - glo]
    return out


_CACHE = {}


def kernel(**inputs):
    from concourse.bass_utils import run_bass_kernel_spmd
    meta, shared, per_core = preprocess(inputs)
    key = (meta["NBLK"], meta["NCB"])
    if key not in _CACHE:
        _CACHE[key] = build_program(meta, verbose=True)
    nc = _CACHE[key]
    in_maps = make_in_maps(meta, shared, per_core)
    r = run_bass_kernel_spmd(nc, in_maps, list(range(meta["ncores"])))
    return assemble_output(meta, per_core, r.results)

